# revision 8
# baseline (speedup 1.0000x reference)
"""Trainium2 Bass kernel for nn_Classifier (segment_reduce), bf16 edition.

Computation (reference semantics):
  attn  = concat(emb, pos) @ W_attn + b_attn          (S, T, 1)
  w     = softmax(attn, axis=1)                        per-segment over T
  segv  = sum_t w * emb                                (S, BERT)
  vecs  = segment_sum(segv, segment_ids, 64)           (64, BERT)
  out   = sigmoid(lrelu(lrelu(vecs@W1+b1)@W2+b2)@W3+b3)

Sharding: data-parallel over S across 8 NeuronCores (32 segments each),
AllReduce of the feat-major (768, 64) comment partials, replicated MLP.

Performance notes:
 - All bulk tensors staged to HBM as bf16 (host-side cast): halves DMA
   bytes, 1 cyc/row matmuls, smaller collective.
 - emb+pos packed host-side into one [128, sl, nt*feat] tensor: each
   segment is ONE dma_start, per-partition contiguous 7168B.
 - Logit path (ep/wab/prod/E) uses fp16 (better DVE behavior than bf16
   measured on HW); everything else bf16. gpsimd offload of the logit
   STT is disabled: it breaks neuronxcc walrus lowering (GP_SPLIT).
 - Comment partials are built feat-major (cmT = segvecs^T @ oneh_sc) so
   the MLP's first layer needs no transposes after the AllReduce.
 - b_attn shifts all logits of a segment equally -> softmax-invariant.
 - softmax 1/den folded into the one-hot segment->comment matrix.
"""

import sys

sys.path.insert(0, "/opt/trn_rl_repo")

import numpy as np
import ml_dtypes

BF = ml_dtypes.bfloat16

S, T, BERT, POS = 256, 512, 768, 128
FEAT = BERT + POS
H1 = 1024
NCLS = 6
NCOM = 64
NCORES = 8
GP_SPLIT = False  # gpsimd scalar_tensor_tensor breaks neuronxcc walrus lowering

_CACHE = {}


def build_nc(n_cores, sl, t, bert, pos, h1, ncls, ncom):
    """Build the SPMD Bass program for one core (sl segments/core)."""
    import concourse.bass as bass
    import concourse.mybir as mybir
    import concourse.tile as tile
    from concourse import bacc
    from concourse.masks import make_identity

    f32 = mybir.dt.float32
    bf16 = mybir.dt.bfloat16
    fp16 = mybir.dt.float16
    AF = mybir.ActivationFunctionType
    OP = mybir.AluOpType
    AX = mybir.AxisListType

    feat = bert + pos
    nt = t // 128
    nk1 = bert // 128
    nk2 = h1 // 128
    BLK = 8  # segments per den block

    nc = bacc.Bacc(
        "TRN2", target_bir_lowering=False, debug=False, num_devices=n_cores
    )

    ep_d = nc.dram_tensor("ep", [128, sl, nt * feat], fp16, kind="ExternalInput").ap()
    wab_d = nc.dram_tensor("wab", [128, feat], fp16, kind="ExternalInput").ap()
    oneh_d = nc.dram_tensor("oneh", [sl, ncom], f32, kind="ExternalInput").ap()
    w1_d = nc.dram_tensor("w1", [bert, h1], bf16, kind="ExternalInput").ap()
    b1_d = nc.dram_tensor("b1", [1, h1], bf16, kind="ExternalInput").ap()
    w2_d = nc.dram_tensor("w2", [h1, h1], bf16, kind="ExternalInput").ap()
    b2_d = nc.dram_tensor("b2", [1, h1], bf16, kind="ExternalInput").ap()
    w3_d = nc.dram_tensor("w3", [h1, ncls], bf16, kind="ExternalInput").ap()
    b3_d = nc.dram_tensor("b3", [1, ncls], bf16, kind="ExternalInput").ap()
    out_d = nc.dram_tensor("out", [ncom, ncls], f32, kind="ExternalOutput").ap()

    with tile.TileContext(nc) as tc:
        with (
            tc.tile_pool(name="const", bufs=1) as const_pool,
            tc.tile_pool(name="ep", bufs=10) as ep_pool,
            tc.tile_pool(name="work", bufs=1) as work,
            tc.tile_pool(name="psv", bufs=2, space="PSUM") as psv,
            tc.tile_pool(name="pmisc", bufs=2, space="PSUM") as pmisc,
            tc.tile_pool(name="dram", bufs=1, space="DRAM") as dram,
        ):
            # ---- constants ----
            wab_sb = const_pool.tile([128, feat], fp16)
            nc.sync.dma_start(wab_sb, wab_d)
            oneh_sb = const_pool.tile([sl, ncom], f32)
            nc.sync.dma_start(oneh_sb, oneh_d)
            identity = const_pool.tile([128, 128], bf16)
            make_identity(nc, identity)
            ones_sb = const_pool.tile([128, 64], bf16)
            nc.gpsimd.memset(ones_sb, 1.0)

            # ---- persistent working tiles ----
            L_sb = work.tile([128, nt * sl], f32)      # logits, col = s*nt + i
            E_sb = work.tile([128, nt * sl], fp16)     # exp(logits)
            segvecs = work.tile([sl, bert], bf16)      # unnormalized segvecs
            den_sb = work.tile([1, sl], f32)           # sum_t e per segment

            # ---- main loop over local segments ----
            for s in range(sl):
                ep = ep_pool.tile([128, nt * feat], fp16, tag="ep")
                nc.sync.dma_start(ep, ep_d[:, s])
                # attention logits: fused multiply + free-dim reduce
                for i in range(nt):
                    on_gp = GP_SPLIT and i == nt - 1
                    eng = nc.gpsimd if on_gp else nc.vector
                    prod = ep_pool.tile(
                        [128, feat], fp16, tag="prodg" if on_gp else "prod", bufs=2
                    )
                    eng.scalar_tensor_tensor(
                        prod,
                        ep[:, i * feat : (i + 1) * feat],
                        1.0,
                        wab_sb,
                        op0=OP.mult,
                        op1=OP.mult,
                        accum_out=L_sb[:, nt * s + i : nt * s + i + 1],
                    )
                # e = exp(logits), downcast to bf16 for the pooling matmul
                nc.scalar.activation(
                    E_sb[:, nt * s : nt * s + nt],
                    L_sb[:, nt * s : nt * s + nt],
                    AF.Exp,
                )
                # pooling: segvec[s] = E-weighted sum over tokens
                sv = psv.tile([1, bert], f32, tag="sv")
                for i in range(nt):
                    col = nt * s + i
                    for n0 in range(0, bert, 512):
                        n1 = min(n0 + 512, bert)
                        nc.tensor.matmul(
                            sv[0:1, n0:n1],
                            E_sb[:, col : col + 1],
                            ep[:, i * feat + n0 : i * feat + n1],
                            start=(i == 0),
                            stop=(i == nt - 1),
                        )
                stage = work.tile([1, bert], bf16, tag="stage", bufs=3, name="stage")
                nc.scalar.copy(stage, sv)
                nc.sync.dma_start(segvecs[s : s + 1, :], stage)

                # den for a finished block of segments (keeps the end-of-loop
                # critical path to one small block)
                if s % BLK == BLK - 1:
                    b0 = s - (BLK - 1)
                    den_blk = pmisc.tile([1, BLK * nt], f32, tag="m", name="den")
                    nc.tensor.matmul(
                        den_blk,
                        ones_sb[:, 0:1],
                        E_sb[:, nt * b0 : nt * (s + 1)],
                        start=True,
                        stop=True,
                    )
                    nc.vector.tensor_reduce(
                        den_sb[:, b0 : s + 1],
                        den_blk.rearrange("p (s i) -> p s i", i=nt),
                        axis=AX.X,
                        op=OP.add,
                    )

            # ---- MLP weights: loaded late so they don't delay ep tiles ----
            b1_sb = const_pool.tile([1, h1], bf16)
            nc.sync.dma_start(b1_sb, b1_d)
            b2_sb = const_pool.tile([1, h1], bf16)
            nc.sync.dma_start(b2_sb, b2_d)
            b3_sb = const_pool.tile([1, ncls], bf16)
            nc.sync.dma_start(b3_sb, b3_d)
            w1_sb = const_pool.tile([128, nk1, h1], bf16)
            nc.sync.dma_start(w1_sb, w1_d.rearrange("(j p) h -> p j h", p=128))
            w2_sb = const_pool.tile([128, nk2, h1], bf16)
            nc.sync.dma_start(w2_sb, w2_d.rearrange("(j p) h -> p j h", p=128))
            w3_sb = const_pool.tile([128, nk2, ncls], bf16)
            nc.sync.dma_start(w3_sb, w3_d.rearrange("(j p) h -> p j h", p=128))

            # ---- 1/den, folded into the one-hot ----
            den_bf = work.tile([1, sl], bf16)
            nc.vector.tensor_copy(den_bf, den_sb)
            den_col = pmisc.tile([sl, 1], f32, tag="m")
            nc.tensor.matmul(
                den_col, den_bf, ones_sb[0:1, 0:1], start=True, stop=True
            )
            inv_den = work.tile([sl, 1], f32)
            nc.vector.reciprocal(inv_den, den_col)
            oneh_sc = work.tile([sl, ncom], bf16)
            nc.vector.tensor_scalar_mul(oneh_sc, oneh_sb, inv_den)

            # ---- feat-major comment partials: cmT = segvecs^T @ oneh_sc ----
            cmT = pmisc.tile([128, nk1, ncom], f32, tag="m")
            for j in range(nk1):
                nc.tensor.matmul(
                    cmT[:, j, :],
                    segvecs[:, 128 * j : 128 * (j + 1)],
                    oneh_sc,
                    start=True,
                    stop=True,
                )
            partialT = work.tile([128, nk1 * ncom], bf16)
            nc.scalar.copy(partialT, cmT.rearrange("p j c -> p (j c)"))

            # ---- AllGather + local tree-reduce (one ring phase instead of
            # the AllReduce's two; the 8-way add costs ~3us of idle DVE) ----
            ar_in = dram.tile([128, nk1 * ncom], bf16)
            ag_out = dram.tile([n_cores, 128, nk1 * ncom], bf16)
            nc.sync.dma_start(ar_in, partialT)
            nc.gpsimd.collective_compute(
                "AllGather",
                OP.bypass,
                replica_groups=[list(range(n_cores))],
                ins=[ar_in.opt()],
                outs=[ag_out.opt()],
            )
            gath = work.tile([128, n_cores, nk1 * ncom], bf16)
            nc.sync.dma_start(gath, ag_out.rearrange("c p f -> p c f"))
            r4 = work.tile([128, 4, nk1 * ncom], f32)
            nc.vector.tensor_tensor(
                r4, gath[:, 0:4, :], gath[:, 4:8, :], op=OP.add
            )
            r2 = work.tile([128, 2, nk1 * ncom], f32)
            nc.vector.tensor_tensor(r2, r4[:, 0:2, :], r4[:, 2:4, :], op=OP.add)
            vecsT = work.tile([128, nk1, ncom], bf16)
            nc.vector.tensor_tensor(
                vecsT.rearrange("p j c -> p (j c)"),
                r2[:, 0, :],
                r2[:, 1, :],
                op=OP.add,
            )

            # ---- MLP (replicated on every core) ----
            def layer(xT, nk, ndim, w_sb, b_sb, act):
                """xT: feat-major input [128, nk, ncom]; returns com-major y."""
                h_ps = pmisc.tile([ncom, ndim], f32, tag="m", name="h_ps")
                for n0 in range(0, ndim, 512):
                    n1 = min(n0 + 512, ndim)
                    for j in range(nk):
                        nc.tensor.matmul(
                            h_ps[:, n0:n1],
                            xT[:, j, :],
                            w_sb[:, j, n0:n1],
                            start=(j == 0),
                            stop=False,
                        )
                    nc.tensor.matmul(
                        h_ps[:, n0:n1],
                        ones_sb[0:1, 0:ncom],
                        b_sb[:, n0:n1],
                        start=False,
                        stop=True,
                    )
                if act == "lrelu":
                    y_sb = work.tile([ncom, ndim], bf16, tag="y", name="y_sb")
                    x_st = work.tile([ncom, ndim], bf16, tag="xs", name="x_st")
                    nc.scalar.copy(x_st, h_ps)
                    nc.vector.scalar_tensor_tensor(
                        y_sb, x_st, 0.01, x_st, op0=OP.mult, op1=OP.max
                    )
                else:
                    y_sb = work.tile([ncom, ndim], f32, tag="yf", name="y_f32")
                    nc.scalar.activation(y_sb, h_ps, AF.Sigmoid)
                return y_sb

            def transpose_in(x_sb, nk):
                """com-major (ncom, nk*128) -> feat-major [128, nk, ncom]."""
                xT = work.tile([128, nk, ncom], bf16, tag="xT", name="xT")
                for j in range(nk):
                    tp2 = pmisc.tile([128, ncom], bf16, tag="m", name="tp2")
                    nc.tensor.transpose(
                        tp2,
                        x_sb[:, 128 * j : 128 * (j + 1)],
                        identity[0:ncom, 0:ncom],
                    )
                    nc.vector.tensor_copy(xT[:, j, :], tp2)
                return xT

            h1_sb = layer(vecsT, nk1, h1, w1_sb, b1_sb, "lrelu")
            h1T = transpose_in(h1_sb, nk2)
            h2_sb = layer(h1T, nk2, h1, w2_sb, b2_sb, "lrelu")
            h2T = transpose_in(h2_sb, nk2)
            y_sb = layer(h2T, nk2, ncls, w3_sb, b3_sb, "sigmoid")

            nc.sync.dma_start(out_d, y_sb)

    nc.compile()
    return nc


def make_in_maps(
    embeddings,
    position_encodings,
    W_attn,
    W1,
    b1,
    W2,
    b2,
    W3,
    b3,
    segment_ids,
    n_cores,
    ncom,
):
    """Host-side sharding: slice S across cores, pack emb+pos into the
    [128, sl, nt*feat] bf16 layout, build per-core one-hot."""
    f32 = np.float32
    s_total = embeddings.shape[0]
    sl = s_total // n_cores
    t = embeddings.shape[1]
    bert = embeddings.shape[2]
    pos = position_encodings.shape[2]
    feat = bert + pos
    nt = t // 128

    wa = np.asarray(W_attn, dtype=f32).reshape(-1)
    wab = np.ascontiguousarray(np.tile(wa[None, :], (128, 1))).astype(np.float16)

    embb = np.asarray(embeddings, dtype=f32).astype(np.float16)
    posb = np.asarray(position_encodings, dtype=f32).astype(np.float16)

    seg = np.asarray(segment_ids).astype(np.int64).reshape(-1)
    common = {
        "wab": wab,
        "w1": np.asarray(W1, dtype=f32).astype(BF),
        "b1": np.asarray(b1, dtype=f32).reshape(1, -1).astype(BF),
        "w2": np.asarray(W2, dtype=f32).astype(BF),
        "b2": np.asarray(b2, dtype=f32).reshape(1, -1).astype(BF),
        "w3": np.asarray(W3, dtype=f32).astype(BF),
        "b3": np.asarray(b3, dtype=f32).reshape(1, -1).astype(BF),
    }
    in_maps = []
    for c in range(n_cores):
        oneh = np.zeros((sl, ncom), dtype=f32)
        local = seg[c * sl : (c + 1) * sl]
        oneh[np.arange(sl), local] = 1.0
        e = embb[c * sl : (c + 1) * sl].reshape(sl, nt, 128, bert)
        p = posb[c * sl : (c + 1) * sl].reshape(sl, nt, 128, pos)
        ep = np.empty((128, sl, nt, feat), dtype=np.float16)
        ep[:, :, :, :bert] = e.transpose(2, 0, 1, 3)
        ep[:, :, :, bert:] = p.transpose(2, 0, 1, 3)
        in_maps.append(
            {
                "ep": ep.reshape(128, sl, nt * feat),
                "oneh": oneh,
                **common,
            }
        )
    return in_maps


def kernel(
    embeddings,
    position_encodings,
    W_attn,
    b_attn,
    W1,
    b1,
    W2,
    b2,
    W3,
    b3,
    segment_ids,
    num_comments,
):
    from concourse.bass_utils import run_bass_kernel_spmd

    assert int(num_comments) == NCOM
    assert embeddings.shape == (S, T, BERT)
    assert position_encodings.shape == (S, T, POS)

    key = "full"
    if key not in _CACHE:
        _CACHE[key] = build_nc(NCORES, S // NCORES, T, BERT, POS, H1, NCLS, NCOM)
    nc = _CACHE[key]

    in_maps = make_in_maps(
        embeddings,
        position_encodings,
        W_attn,
        W1,
        b1,
        W2,
        b2,
        W3,
        b3,
        segment_ids,
        NCORES,
        NCOM,
    )
    res = run_bass_kernel_spmd(nc, in_maps, list(range(NCORES)))
    return np.asarray(res.results[0]["out"], dtype=np.float32)
